# revision 1
# baseline (speedup 1.0000x reference)
"""CenterLoss kernel for Trainium2 (8 NeuronCores, SPMD data-parallel).

Reference computes
    distmat[b,c] = ||x_b||^2 + ||c_c||^2 - 2<x_b, c_c>          [B, C]
    loss = sum(clip(distmat * onehot(labels), 1e-12, 1e12)) / B

Only distmat[b, labels[b]] survives the mask; each of the B*(C-1) masked
zeros becomes exactly 1e-12 under the clip. So instead of the [8192, 10000]
distmat (42 GFLOP), each core gathers its rows' centers with indirect DMA
and computes per-row squared distances; the host adds the closed-form
constant B*(C-1)*1e-12 and divides by B.

Sharding: batch split 8 ways (1024 rows/core), centers replicated.

Per-core kernel (all stock ISA):
  - one [128, 8] int32 idx tile (labels, row p*8+g at [p, g])
  - one contiguous 1MB x load as [128, 8, 256] (row p*8+g at [p, g, :])
  - 8 indirect row-gathers (Q7 SWDGE, 128 rows each) whose offset APs are
    column slices of the idx tile; compute pipelined behind them:
    DVE subtract, ACT Square with accum_out giving the row reduction.
  - [128, 8] partial distances DMA'd out; host clamps at 1e-12 and sums.

Hard-won HW constraints baked in here (this runtime rejects/crashes
otherwise):
  - Use Bacc, and finalize() before run: TRN2 codegen allows ONE sync-wait
    per instruction; Bacc's generate_event_semaphores splits multi-waits,
    and the bass2jax path serializes the module without finalizing.
  - Stock instructions only: custom "Ant" ISA ops (tensor_tensor_reduce,
    dma_gather, ...) kill the exec unit (NRT_EXEC_UNIT_UNRECOVERABLE).
  - No in-place DVE ops (out aliasing an input) — same crash.
  - indirect_dma_start: offset AP may be a [128, 1] column slice, but the
    dest must be a whole [128, D] tile; multi-column offsets or strided
    dest slices gather garbage.
"""

import numpy as np

from concourse import bacc, bass, mybir
import concourse.tile as tile
from concourse.bass_utils import run_bass_kernel_spmd

B = 8192
C = 10000
D = 256
N_CORES = 8
BL = B // N_CORES  # rows per core
P = 128            # SBUF partitions
G = BL // P        # row groups per core

_CLIP_LO = 1e-12

_nc_cache = None


def _build():
    global _nc_cache
    if _nc_cache is not None:
        return _nc_cache

    nc = bacc.Bacc()
    x_l = nc.dram_tensor("x_local", [BL, D], mybir.dt.float32, kind="ExternalInput")
    lab_l = nc.dram_tensor("labels_local", [BL], mybir.dt.int32, kind="ExternalInput")
    cen = nc.dram_tensor("centers", [C, D], mybir.dt.float32, kind="ExternalInput")
    out = nc.dram_tensor("partials", [P, G], mybir.dt.float32, kind="ExternalOutput")

    with tile.TileContext(nc) as tc:
        with (
            tc.tile_pool(name="big", bufs=1) as big,
            tc.tile_pool(name="work", bufs=4) as work,
            # gather dests get all 8 slots: late gathers then never carry a
            # slot-release wait, keeping the Q7 chain free of EVSEM stalls
            tc.tile_pool(name="ctp", bufs=G) as ctp,
        ):
            lt = big.tile([P, G], mybir.dt.int32)
            xt = big.tile([P, G, D], mybir.dt.float32)
            acc = big.tile([P, G], mybir.dt.float32)

            # idx tile first: the whole gather chain hangs off it
            nc.sync.dma_start(out=lt[:], in_=lab_l[:].rearrange("(p g) -> p g", g=G))
            # x in halves so early groups aren't gated on the full 1MB
            x_ap = x_l[:].rearrange("(p g) d -> p g d", g=G)
            nc.sync.dma_start(out=xt[:, 0:G // 2, :], in_=x_ap[:, 0:G // 2, :])
            nc.sync.dma_start(out=xt[:, G // 2:, :], in_=x_ap[:, G // 2:, :])

            for g in range(G):
                ct = ctp.tile([P, D], mybir.dt.float32, tag="ct")
                nc.gpsimd.indirect_dma_start(
                    out=ct[:],
                    out_offset=None,
                    in_=cen[:],
                    in_offset=bass.IndirectOffsetOnAxis(ap=lt[:, g:g + 1], axis=0),
                )
                dt = work.tile([P, D], mybir.dt.float32, tag="dt")
                nc.vector.tensor_sub(out=dt[:], in0=xt[:, g, :], in1=ct[:])
                sq = work.tile([P, D], mybir.dt.float32, tag="sq")
                nc.scalar.activation(
                    out=sq[:],
                    in_=dt[:],
                    func=mybir.ActivationFunctionType.Square,
                    accum_out=acc[:, g:g + 1],
                )
            nc.sync.dma_start(out=out[:], in_=acc[:])

    nc.finalize()
    _nc_cache = nc
    return nc


def _run(x, labels, centers, **spmd_kwargs):
    nc = _build()
    x = np.ascontiguousarray(np.asarray(x), dtype=np.float32)
    labels = np.ascontiguousarray(np.asarray(labels)).astype(np.int32)
    centers = np.ascontiguousarray(np.asarray(centers), dtype=np.float32)

    in_maps = []
    for c in range(N_CORES):
        sl = slice(c * BL, (c + 1) * BL)
        in_maps.append(
            {
                "x_local": x[sl],
                "labels_local": labels[sl],
                "centers": centers,
            }
        )
    res = run_bass_kernel_spmd(nc, in_maps, list(range(N_CORES)), **spmd_kwargs)
    partials = np.stack([r["partials"] for r in res.results])  # [8, P, G]
    clamped = np.maximum(partials.astype(np.float64), _CLIP_LO)
    loss = (clamped.sum() + B * (C - 1) * _CLIP_LO) / B
    return np.asarray(loss, dtype=np.float32), res


def kernel(x, labels, centers):
    loss, _ = _run(x, labels, centers)
    return loss



# revision 2
# speedup vs baseline: 1.0895x; 1.0895x over previous
"""CenterLoss kernel for Trainium2 (8 NeuronCores, SPMD).

Reference computes
    distmat[b,c] = ||x_b||^2 + ||c_c||^2 - 2<x_b, c_c>          [B, C]
    loss = sum(clip(distmat * onehot(labels), 1e-12, 1e12)) / B

Only distmat[b, labels[b]] survives the mask, so the loss needs
d_b = ||x_b - centers[l_b]||^2 per row, plus the closed-form constant
B*(C-1)*1e-12 for the masked zeros the clip turns into 1e-12.

Architecture (chosen after tracing the gather-everything baseline, which
is hard-floored at ~11us by Q7 SWDGE descriptor emission: 1024 indirect
rows/core x ~8.5ns/descriptor, 128 rows max per call):

  Class-sharded centers + aligned x placement (the sharding_hint's
  "shard column-wise over num_classes"), batch rows routed to the core
  owning their label (host-side index work only):

  - core k owns classes [1250k, 1250(k+1)), padded to 1280 = 128x10
    slots; its shard loads as ONE contiguous DMA (no descriptors/index).
  - the FIRST row of each distinct class is placed at its class's slot
    in an aligned x buffer, interleaved [c|x] per (partition, slot) so
    each 5-slot half is a single 5KB-per-partition-run DMA with one
    completion receipt.
  - duplicate-class rows (seed-0 max 351/core) go to a 384-slot overflow
    handled by 3 classic indirect gathers (vs 8 in the baseline).
  - sum-collapse trick: the host only ever SUMS per-row distances
    (clip(d,1e-12) is a no-op for real rows, d ~ chi^2(256)*2 ~ 512), so
    empty aligned slots are padded with x := c (bf16-identical values
    subtract to exactly 0) and empty overflow slots with x := centers[0]
    (the row jo=0 gathers). Every pad contributes exactly 0 and each
    region half reduces on-device to [128,1] via one ACT Square+accum;
    overflow reduces via DVE sub+mul+reduce. Output is just [128,5]/core.
  - bf16 data path (f32 accumulation): rel err ~1e-5, tolerance is 2e-2.
  - an explicit same-engine ordering dep keeps the in-order DVE from
    hoisting overflow work ahead of the region-b subtract (the Tile cost
    model underestimates gather completion latency by ~2.5us).

  Rows beyond a core's region+overflow capacity (impossible for uniform
  labels; the harness input peaks at 351 of 384) spill to additional
  identical invocations, preserving correctness for any input.

Measured: ~22.6-23.5us vs 28.0-28.6us for the staged baseline.

Hard-won HW constraints baked in (this runtime rejects/crashes otherwise):
  - Bacc + finalize() before run; stock instructions only (custom "Ant"
    ISA ops kill the exec unit); no in-place DVE ops.
  - indirect_dma_start: offset AP must be a [128, 1] column, dest a whole
    [128, D] tile; multi-column offsets / partition-sliced dests gather
    garbage (verified: ucode reads wrong index positions).
  - Q7 emission is per-descriptor (~8.5ns) regardless of element size.
  - SWDGE gather data drains only after the concurrent HWDGE backlog and
    its completion semaphore lags the last packet by ~1.5-2.5us.
"""

import numpy as np

from concourse import bacc, bass, mybir
import concourse.tile as tile
from concourse.bass_utils import run_bass_kernel_spmd

B = 8192
C = 10000
D = 256
N_CORES = 8
P = 128

CPC = C // N_CORES          # classes per core (1250)
S = (CPC + P - 1) // P      # region slots per partition (10)
CPAD = P * S                # padded classes per core (1280)
OVF = 384                   # overflow rows per core (seed-0 max is 351)
J = OVF // P                # overflow gather calls (3)
H = S // 2                  # region half (5 slots)

_CLIP_LO = 1e-12

_nc_cache = None


def _build():
    global _nc_cache
    if _nc_cache is not None:
        return _nc_cache

    nc = bacc.Bacc()
    # cx*: [c|x] interleaved per (partition, slot) -> 5KB contiguous
    # per-partition runs, one completion receipt per half.
    cxa = nc.dram_tensor("cxa", [P, H, 2, D], mybir.dt.bfloat16, kind="ExternalInput")
    cxb = nc.dram_tensor("cxb", [P, H, 2, D], mybir.dt.bfloat16, kind="ExternalInput")
    xo = nc.dram_tensor("xovf", [OVF, D], mybir.dt.bfloat16, kind="ExternalInput")
    jo = nc.dram_tensor("jovf", [OVF], mybir.dt.int32, kind="ExternalInput")
    cen = nc.dram_tensor("centers", [C, D], mybir.dt.bfloat16, kind="ExternalInput")
    out = nc.dram_tensor("sums", [P, 2 + J], mybir.dt.float32, kind="ExternalOutput")

    with tile.TileContext(nc) as tc:
        with (
            tc.tile_pool(name="big", bufs=1) as big,
            tc.tile_pool(name="work", bufs=4) as work,
            tc.tile_pool(name="gtp", bufs=J) as gtp,
        ):
            jt = big.tile([P, J], mybir.dt.int32)
            cxat = big.tile([P, H, 2, D], mybir.dt.bfloat16)
            cxbt = big.tile([P, H, 2, D], mybir.dt.bfloat16)
            xot = big.tile([P, J, D], mybir.dt.bfloat16)
            acc = big.tile([P, 2 + J], mybir.dt.float32)

            # SP ring FIFO: jt -> cxa -> cxb (arrival-ordered, per-ring FIFO
            # completes early loads early); xot parallel on the ACT ring.
            nc.sync.dma_start(out=jt[:], in_=jo[:].rearrange("(p j) -> p j", j=J))
            nc.sync.dma_start(out=cxat[:], in_=cxa[:])
            nc.sync.dma_start(out=cxbt[:], in_=cxb[:])
            nc.scalar.dma_start(
                out=xot[:], in_=xo[:].rearrange("(p j) d -> p j d", j=J)
            )

            gts = []
            for j in range(J):
                gt = gtp.tile([P, D], mybir.dt.bfloat16, tag="gt")
                nc.gpsimd.indirect_dma_start(
                    out=gt[:],
                    out_offset=None,
                    in_=cen[:],
                    in_offset=bass.IndirectOffsetOnAxis(ap=jt[:, j:j + 1], axis=0),
                )
                gts.append(gt)

            def region(cxt, col, tag):
                dtc = work.tile([P, H, D], mybir.dt.bfloat16, tag=f"dt{tag}")
                sub_inst = nc.vector.tensor_sub(
                    out=dtc[:], in0=cxt[:, :, 1, :], in1=cxt[:, :, 0, :]
                )
                sqc = work.tile([P, H, D], mybir.dt.bfloat16, tag=f"sq{tag}")
                nc.scalar.activation(
                    out=sqc[:],
                    in_=dtc[:],
                    func=mybir.ActivationFunctionType.Square,
                    accum_out=acc[:, col:col + 1],
                )
                return sub_inst

            def ovf(j, after=None):
                dt = work.tile([P, D], mybir.dt.bfloat16, tag="dto")
                sub_inst = nc.vector.tensor_sub(
                    out=dt[:], in0=xot[:, j, :], in1=gts[j][:]
                )
                if after is not None:
                    tile.add_dep_helper(
                        sub_inst.ins,
                        after.ins,
                        sync=False,
                        reason="keep region subs ahead of overflow on DVE",
                    )
                sq = work.tile([P, D], mybir.dt.bfloat16, tag="sqo")
                nc.vector.tensor_mul(out=sq[:], in0=dt[:], in1=dt[:])
                nc.vector.tensor_reduce(
                    out=acc[:, 2 + j:3 + j],
                    in_=sq[:],
                    axis=mybir.AxisListType.X,
                    op=mybir.AluOpType.add,
                )

            region(cxat, 0, "a")
            sub_b = region(cxbt, 1, "b")
            ovf(0, after=sub_b)
            ovf(1)
            ovf(2)

            nc.sync.dma_start(out=out[:], in_=acc[:])

    nc.finalize()
    _nc_cache = nc
    return nc


def _pack_core(x_bf, cs_k, cen0, lab, rows, k):
    """Pack one core's rows. Empty region slots get x:=c (contributes 0);
    empty overflow slots get x:=centers[0], jo:=0 (contributes 0)."""
    m = lab[rows] - k * CPC
    order = np.argsort(m, kind="stable")
    rs = rows[order]
    ms = m[order]
    first = np.ones(len(ms), dtype=bool)
    first[1:] = ms[1:] != ms[:-1]

    xr = cs_k.copy()  # empty slots: x == c -> exactly 0
    xr[ms[first]] = x_bf[rs[first]]

    rest = rs[~first]
    ovf_rows = rest[:OVF]
    leftover = rest[OVF:]
    xo = np.broadcast_to(cen0, (OVF, D)).copy()
    jo = np.zeros(OVF, dtype=np.int32)
    xo[: len(ovf_rows)] = x_bf[ovf_rows]
    jo[: len(ovf_rows)] = lab[ovf_rows]
    return xr, xo, jo, leftover


def _run(x, labels, centers, **spmd_kwargs):
    import jax.numpy as jnp

    nc = _build()
    x = np.ascontiguousarray(np.asarray(x), dtype=np.float32)
    labels = np.ascontiguousarray(np.asarray(labels)).astype(np.int64)
    centers = np.ascontiguousarray(np.asarray(centers), dtype=np.float32)

    bf = jnp.bfloat16
    x_bf = np.asarray(jnp.asarray(x, dtype=bf))
    cen_bf = np.asarray(jnp.asarray(centers, dtype=bf))
    cs_pad = np.zeros((N_CORES, CPAD, D), dtype=cen_bf.dtype)
    cs_pad[:, :CPC] = cen_bf.reshape(N_CORES, CPC, D)

    owner = labels // CPC
    pending = [np.flatnonzero(owner == k) for k in range(N_CORES)]

    total = 0.0
    res = None
    while any(len(r) for r in pending):
        in_maps = []
        next_pending = []
        for k in range(N_CORES):
            xr, xo, jo, leftover = _pack_core(
                x_bf, cs_pad[k], cen_bf[0], labels, pending[k], k
            )
            cs3 = cs_pad[k].reshape(P, S, D)
            xr3 = xr.reshape(P, S, D)
            in_maps.append(
                {
                    "cxa": np.ascontiguousarray(
                        np.stack([cs3[:, :H], xr3[:, :H]], axis=2)
                    ),
                    "cxb": np.ascontiguousarray(
                        np.stack([cs3[:, H:], xr3[:, H:]], axis=2)
                    ),
                    "xovf": xo,
                    "jovf": jo,
                    "centers": cen_bf,
                }
            )
            next_pending.append(leftover)
        res = run_bass_kernel_spmd(nc, in_maps, list(range(N_CORES)), **spmd_kwargs)
        for k in range(N_CORES):
            total += res.results[k]["sums"].astype(np.float64).sum()
        pending = next_pending
        spmd_kwargs = {}  # only trace the first invocation

    loss = (total + B * (C - 1) * _CLIP_LO) / B
    return np.asarray(loss, dtype=np.float32), res


def kernel(x, labels, centers):
    loss, _ = _run(x, labels, centers)
    return loss
